# revision 11
# baseline (speedup 1.0000x reference)
"""Trainium2 Bass kernel for a small decoder block (nn_Decoder_75849122448079).

Math (N=4096 seq, W=512 width, P=64 proj, H=8 heads, F=2048 ffn):
  masked_mh = softmax(q_m k_m^T / 8) v_m @ w_o_sum      (w_o_sum = sum of H row-blocks of w_o)
  mh        = softmax(q_c k_c^T / 8) v_c @ w_o_sum      (q_c from masked_mh; k_c/v_c from x)
  h   = LN(mh + x) * g + b
  y   = LeakyReLU(h @ w1 + b1) @ w2 + b2
  out = LN(y + h) * g + b

Sharding: data-parallel over sequence rows — each of the 8 cores owns 512 query
rows end-to-end. K/V projections are computed on each core's own row slice and
exchanged with a single packed AllGather; everything else is local. The host
only slices x, casts weights to the compute dtype, and concatenates outputs.

Attention trick: scores are built transposed (S^T[k, q] = K Q^T), exp'd on the
ACT engine straight out of PSUM, and the softmax denominator rides along as a
ones-column appended to V, so no partition-axis reduction is ever needed. The
masked branch normalizes A in [q, d] layout; the cross branch defers its
normalization through the w_o_sum matmul into the residual step.
"""

import os

import numpy as np

import concourse.bass as bass
import concourse.bacc as bacc
import concourse.mybir as mybir
import concourse.tile as tile
from concourse.bass_utils import run_bass_kernel_spmd
from concourse.masks import make_identity

N, W, P, H, F = 4096, 512, 64, 8, 2048
NCORES = 8
R = N // NCORES          # 512 rows per core
RT = R // 128            # 4 row tiles per core
WC = W // 128            # 4 contraction chunks over width
ST = N // 128            # 32 sequence (key) tiles
FC = F // 128            # 16 ffn-hidden tiles
EPS = 1e-5
LEAKY = 0.01
SCALE = 0.125            # 1/sqrt(P)
SLOT = (P * R)           # 32768 elements per allgather slot

f32 = mybir.dt.float32
bf16 = mybir.dt.bfloat16

# Compute dtype mode: "f32" (exact, 4 cyc/row) or "bf16" (fast, ~1e-3 rel err).
MODE = os.environ.get("BASS_DECODER_MODE", "bf16")

# weights cast to the compute dtype host-side in bf16 mode
CAST_NAMES = ["w_q_m", "w_k_m", "w_v_m", "w_q_c", "w_k_c", "w_v_c",
              "w_o", "ffn_w1", "ffn_w2"]


def build_nc(mode=MODE):
    cd = bf16 if mode == "bf16" else f32
    nc = bacc.Bacc()

    spec = [("x_rows", [R, W], f32),
            ("w_q_m", [W, P], cd), ("w_k_m", [W, P], cd), ("w_v_m", [W, P], cd),
            ("w_q_c", [W, P], cd), ("w_k_c", [W, P], cd), ("w_v_c", [W, P], cd),
            ("w_o", [H * P, W], cd),
            ("ffn_w1", [W, F], cd), ("ffn_w2", [F, W], cd),
            ("ln_g", [W], f32), ("ln_b", [W], f32),
            ("ffn_b1", [F], f32), ("ffn_b2", [W], f32)]
    t = {}
    for n, s, d in spec:
        t[n] = nc.declare_dram_parameter(n, s, d, isOutput=False)
    t["out"] = nc.declare_dram_parameter("out", [R, W], f32, isOutput=True)

    with tile.TileContext(nc) as tc:
        _build(tc, mode, cd, t)
    return nc


def _row_bcast(ap, parts=128):
    """AP reading a 1-D DRAM tensor replicated across `parts` partitions."""
    a = ap[:]
    return bass.AP(tensor=a.tensor, offset=a.offset, ap=[[0, parts]] + list(a.ap))


def _build(tc, mode, cd, t):
    nc = tc.nc
    mm = nc.tensor.matmul

    def tp(out, in_, ident):  # PE transpose
        mm(out, in_, ident, is_transpose=True)

    # ------------------------------------------------------------------ pools
    from contextlib import ExitStack
    ctx = ExitStack()
    persist = ctx.enter_context(tc.tile_pool(name="persist", bufs=1))
    stream = ctx.enter_context(tc.tile_pool(name="stream", bufs=2))
    wstream = ctx.enter_context(tc.tile_pool(name="wstream", bufs=3))
    small = ctx.enter_context(tc.tile_pool(name="small", bufs=4))
    pt_pool = ctx.enter_context(tc.tile_pool(name="pt_pool", bufs=3))
    dram = ctx.enter_context(tc.tile_pool(name="dram", bufs=1, space="DRAM"))
    ps_big = ctx.enter_context(tc.tile_pool(name="ps_big", bufs=2, space="PSUM"))
    ps_acc = ctx.enter_context(tc.tile_pool(name="ps_acc", bufs=1, space="PSUM"))

    def big(shape, dtype=f32):
        return ps_big.tile(shape, dtype, tag="big", name="bigtile")

    def acc(shape, dtype=f32):
        return ps_acc.tile(shape, dtype, tag="acc", name="acctile")

    # --------------------------------------------------------- constants
    ident = persist.tile([128, 128], cd)
    make_identity(nc, ident)
    if cd == f32:
        ident_f32 = ident
    else:
        ident_f32 = persist.tile([128, 128], f32)
        make_identity(nc, ident_f32)

    eps_t = persist.tile([128, 1], f32)
    nc.vector.memset(eps_t, EPS)

    g_rep = persist.tile([128, W], f32)
    nc.sync.dma_start(out=g_rep, in_=_row_bcast(t["ln_g"]))
    b_rep = persist.tile([128, W], f32)
    nc.sync.dma_start(out=b_rep, in_=_row_bcast(t["ln_b"]))
    b2_rep = persist.tile([128, W], f32)
    nc.sync.dma_start(out=b2_rep, in_=_row_bcast(t["ffn_b2"]))
    b1_sb = persist.tile([128, FC], f32)
    nc.sync.dma_start(out=b1_sb, in_=t["ffn_b1"].rearrange("(c p) -> p c", p=128))

    # qkv weights; k/v pairs packed on the free dim for merged projections
    wqm = persist.tile([128, WC, P], cd)
    nc.sync.dma_start(out=wqm, in_=t["w_q_m"].rearrange("(c p) d -> p c d", p=128))
    wqc = persist.tile([128, WC, P], cd)
    nc.sync.dma_start(out=wqc, in_=t["w_q_c"].rearrange("(c p) d -> p c d", p=128))
    wk2 = persist.tile([128, WC, 2, P], cd)
    nc.sync.dma_start(out=wk2[:, :, 0, :], in_=t["w_k_m"].rearrange("(c p) d -> p c d", p=128))
    nc.sync.dma_start(out=wk2[:, :, 1, :], in_=t["w_k_c"].rearrange("(c p) d -> p c d", p=128))
    wv2 = persist.tile([128, WC, 2, P], cd)
    nc.sync.dma_start(out=wv2[:, :, 0, :], in_=t["w_v_m"].rearrange("(c p) d -> p c d", p=128))
    nc.sync.dma_start(out=wv2[:, :, 1, :], in_=t["w_v_c"].rearrange("(c p) d -> p c d", p=128))

    # w_o_sum[d, w] = sum_h w_o[h*P + d, w]   -> [64, W]
    wo_stage = stream.tile([64, H, W], cd, tag="wo")
    nc.sync.dma_start(out=wo_stage, in_=t["w_o"].rearrange("(h p) w -> p h w", p=P))
    wos_f32 = persist.tile([64, W], f32)
    nc.vector.tensor_add(wos_f32, wo_stage[:, 0, :], wo_stage[:, 1, :])
    for hh in range(2, H):
        nc.vector.tensor_add(wos_f32, wos_f32, wo_stage[:, hh, :])
    if cd == f32:
        wosum = wos_f32
    else:
        wosum = persist.tile([64, W], cd)
        nc.vector.tensor_copy(wosum, wos_f32)

    # ------------------------------------------------ x_rows (residual + ^T)
    xr_nat = persist.tile([128, RT, W], f32)
    nc.sync.dma_start(out=xr_nat, in_=t["x_rows"].rearrange("(q p) w -> p q w", p=128))
    if cd == f32:
        xr_cd = xr_nat
    else:
        xr_cd = persist.tile([128, RT, W], cd)
        nc.vector.tensor_copy(xr_cd, xr_nat)

    # x_rows^T [128, WC, R]: xrT[p, c, q] = x_rows[q, c*128+p]
    xrT = persist.tile([128, WC, R], cd)
    for qt in range(RT):
        pst = big([128, WC, 128], cd)
        for wc in range(WC):
            tp(pst[:, wc, :], xr_cd[:, qt, wc * 128:(wc + 1) * 128], ident)
        nc.vector.tensor_copy(xrT[:, :, qt * 128:(qt + 1) * 128], pst)

    # ------------------------- own-slice projections  (Q, K-pair, V-pair)
    def proj_T(wt, rhsT, n_free, tag):
        ps = big([64, n_free])
        for wc in range(WC):
            mm(ps, wt[:, wc, :], rhsT[:, wc, :], start=(wc == 0), stop=(wc == WC - 1))
        sb = persist.tile([64, n_free], cd, tag=tag, name=tag)
        nc.vector.tensor_copy(sb, ps)
        return sb

    qmT = proj_T(wqm, xrT, R, "qmT")

    # K^T slices for both attentions in one accumulation: rows 0:64 = masked
    ps_k = big([128, R])
    for wc in range(WC):
        mm(ps_k, wk2[:, wc, :, :], xrT[:, wc, :], start=(wc == 0), stop=(wc == WC - 1))
    km_s = persist.tile([64, R], cd)
    kc_s = persist.tile([64, R], cd)
    nc.vector.tensor_copy(km_s, ps_k[0:64, :])
    nc.vector.tensor_copy(kc_s, ps_k[64:128, :])

    # V slices (natural layout), both attentions per matmul
    vs_m = persist.tile([128, RT, P], cd)
    vs_c = persist.tile([128, RT, P], cd)
    for kt in range(RT):
        ps_v = big([128, 2, P])
        for wc in range(WC):
            mm(ps_v, xrT[:, wc, kt * 128:(kt + 1) * 128], wv2[:, wc, :, :],
               start=(wc == 0), stop=(wc == WC - 1))
        nc.vector.tensor_copy(vs_m[:, kt, :], ps_v[:, 0, :])
        nc.vector.tensor_copy(vs_c[:, kt, :], ps_v[:, 1, :])

    # --------------------------------------------- AllGather of K/V slices
    def allgather(src_sb, out_ap_of_slot):
        b = dram.tile([SLOT], cd, name="bounce", tag=f"b_{src_sb.tensor.name}")
        gth = dram.tile([NCORES, SLOT], cd, addr_space="Shared",
                        name="gath", tag=f"g_{src_sb.tensor.name}")
        nc.sync.dma_start(out=out_ap_of_slot(b), in_=src_sb)
        nc.gpsimd.collective_compute(
            "AllGather", mybir.AluOpType.bypass,
            replica_groups=[list(range(NCORES))],
            ins=[b[:]], outs=[gth[:]],
        )
        return gth

    k_slot = lambda b: b.rearrange("(p s) -> p s", p=64)
    v_slot = lambda b: b.rearrange("(k p d) -> p k d", k=RT, p=128)
    g_km = allgather(km_s, k_slot)
    g_kc = allgather(kc_s, k_slot)
    g_vm = allgather(vs_m, v_slot)
    g_vc = allgather(vs_c, v_slot)

    def g_k_ap(gth):
        g = gth[:]  # [p=64, c=8, s=512]: element at c*SLOT + p*512 + s
        return bass.AP(tensor=g.tensor, offset=g.offset,
                       ap=[[512, 64], [SLOT, NCORES], [1, 512]])

    def g_v_ap(gth):
        g = gth[:]  # [p=128, (c k)=32, d=64]: element at c*SLOT + (k*128+p)*64 + d
        return bass.AP(tensor=g.tensor, offset=g.offset,
                       ap=[[64, 128], [128 * 64, NCORES * RT], [1, 64]])

    kmT = persist.tile([64, ST * 128], cd)
    kcT = persist.tile([64, ST * 128], cd)
    nc.sync.dma_start(out=kmT.rearrange("p (c s) -> p c s", c=NCORES), in_=g_k_ap(g_km))
    nc.sync.dma_start(out=kcT.rearrange("p (c s) -> p c s", c=NCORES), in_=g_k_ap(g_kc))
    vm = persist.tile([128, ST, P + 1], cd)
    vc = persist.tile([128, ST, P + 1], cd)
    nc.vector.memset(vm[:, :, P:P + 1], 1.0)
    nc.vector.memset(vc[:, :, P:P + 1], 1.0)
    nc.sync.dma_start(out=vm[:, :, 0:P], in_=g_v_ap(g_vm))
    nc.sync.dma_start(out=vc[:, :, 0:P], in_=g_v_ap(g_vc))

    # ------------------------------------------------------------- attention
    def attention(kT, v, qT, out_name):
        """A'^T = [v | 1]^T softmax_unnorm(qk^T/8)^T  -> [P+1, R] unnormalized."""
        ps_aT = acc([P + 1, R])
        G = ST // 2

        def scores(g):
            sT = big([128, 2, 512])
            for j in range(2):
                kt = g * 2 + j
                mm(sT[:, j, :], kT[:, kt * 128:(kt + 1) * 128], qT)
            return sT

        sT_prev = scores(0)
        for g in range(1, G + 1):
            sT_next = scores(g) if g < G else None
            ptl = pt_pool.tile([128, 2, 512], cd, tag="pt")
            nc.scalar.activation(ptl, sT_prev, mybir.ActivationFunctionType.Exp,
                                 scale=SCALE)
            for j in range(2):
                kt = (g - 1) * 2 + j
                mm(ps_aT, v[:, kt, :], ptl[:, j, :],
                   start=(kt == 0), stop=(kt == ST - 1))
            sT_prev = sT_next
        aT_sb = persist.tile([P + 1, R], f32, tag=out_name, name=out_name)
        nc.vector.tensor_copy(aT_sb, ps_aT)
        return aT_sb

    # ---------------------------------------------------------- masked branch
    amT = attention(kmT, vm, qmT, "amT")   # [65, R] unnormalized

    # normalize in [q, d] layout: A = A'[:, :64] / A'[:, 64]
    ps_a4 = big([128, RT, P + 1])
    for qt in range(RT):
        tp(ps_a4[:, qt, :], amT[:, qt * 128:(qt + 1) * 128],
           ident_f32[0:P + 1, 0:P + 1])
    a_m = small.tile([128, RT, P], cd, tag="a_m")
    recip_m = small.tile([128, RT, 1], f32, tag="recip")
    for qt in range(RT):
        nc.vector.reciprocal(recip_m[:, qt, :], ps_a4[:, qt, P:P + 1])
        nc.vector.tensor_scalar_mul(a_m[:, qt, :], ps_a4[:, qt, 0:P],
                                    recip_m[:, qt, :])
    # back to A^T [64, R]
    ps_at2 = big([P, R], cd)
    for qt in range(RT):
        tp(ps_at2[:, qt * 128:(qt + 1) * 128], a_m[:, qt, :], ident)
    amT_n = persist.tile([P, R], cd)
    nc.vector.tensor_copy(amT_n, ps_at2)

    # masked_mh^T [128, WC, R] = w_o_sum^T @ A
    mhT = persist.tile([128, WC, R], cd)
    for wc in range(WC):
        ps_mh = big([128, R])
        mm(ps_mh, wosum[:, wc * 128:(wc + 1) * 128], amT_n)
        nc.vector.tensor_copy(mhT[:, wc, :], ps_mh)

    # ----------------------------------------------------------- cross branch
    qcT = proj_T(wqc, mhT, R, "qcT")
    acT = attention(kcT, vc, qcT, "acT")   # [65, R]; row 64 = denominators

    # denominators -> [q, 1] layout, reciprocal
    ps_s1 = big([128, RT, 1])
    for qt in range(RT):
        tp(ps_s1[:, qt, :], acT[P:P + 1, qt * 128:(qt + 1) * 128],
           ident_f32[P:P + 1, P:P + 1])
    rs_c = small.tile([128, RT, 1], f32, tag="rs_c")
    for qt in range(RT):
        nc.vector.reciprocal(rs_c[:, qt, :], ps_s1[:, qt, :])

    if cd == f32:
        acT_cd = acT
    else:
        acT_cd = persist.tile([P + 1, R], cd)
        nc.vector.tensor_copy(acT_cd, acT)

    # ----------------------------------------------- h = LN(mh_c + x) * g + b
    h_f32 = persist.tile([128, RT, W], f32)

    def layer_norm(dst, src):
        """dst = LN(src) * g + b  for [128, W] f32 tiles (may alias)."""
        stats = small.tile([128, 6], f32, tag="stats")
        nc.vector.bn_stats(stats, src)
        mv = small.tile([128, 2], f32, tag="mv")
        nc.vector.bn_aggr(mv, stats)
        nc.scalar.activation(mv[:, 1:2], mv[:, 1:2],
                             mybir.ActivationFunctionType.Sqrt,
                             bias=eps_t, scale=1.0)
        nc.vector.reciprocal(mv[:, 1:2], mv[:, 1:2])
        nc.vector.tensor_scalar(dst, src,
                                scalar1=mv[:, 0:1], scalar2=mv[:, 1:2],
                                op0=mybir.AluOpType.subtract,
                                op1=mybir.AluOpType.mult)
        nc.vector.tensor_mul(dst, dst, g_rep)
        nc.vector.tensor_add(dst, dst, b_rep)

    for qt in range(RT):
        ps_mhc = big([128, W])
        mm(ps_mhc, acT_cd[0:P, qt * 128:(qt + 1) * 128], wosum)
        sum_sb = stream.tile([128, W], f32, tag="sum")
        nc.vector.tensor_scalar_mul(sum_sb, ps_mhc, rs_c[:, qt, :])
        nc.vector.tensor_add(sum_sb, sum_sb, xr_nat[:, qt, :])
        layer_norm(h_f32[:, qt, :], sum_sb)

    if cd == f32:
        h_cd = h_f32
    else:
        h_cd = persist.tile([128, RT, W], cd)
        nc.vector.tensor_copy(h_cd, h_f32)

    # h^T [128, WC, R]
    hT = persist.tile([128, WC, R], cd)
    for qt in range(RT):
        pst = big([128, WC, 128], cd)
        for wc in range(WC):
            tp(pst[:, wc, :], h_cd[:, qt, wc * 128:(wc + 1) * 128], ident)
        nc.vector.tensor_copy(hT[:, :, qt * 128:(qt + 1) * 128], pst)

    # ------------------------------------------------------------------- FFN
    w1_re = t["ffn_w1"].rearrange("(c p) f -> p c f", p=128)   # [128, WC, F]
    w2_re = t["ffn_w2"].rearrange("(c p) w -> p c w", p=128)   # [128, FC, W]

    ps_y2 = acc([128, RT, W])          # one psum bank per row tile
    for fc in range(FC):
        w1_sb = wstream.tile([128, WC, 128], cd, tag="w1_sb")
        nc.sync.dma_start(out=w1_sb, in_=w1_re[:, :, fc * 128:(fc + 1) * 128])
        w2_sb = wstream.tile([128, W], cd, tag="w2_sb")
        nc.sync.dma_start(out=w2_sb, in_=w2_re[:, fc, :])

        ps_y1 = big([128, R])
        for wc in range(WC):
            mm(ps_y1, w1_sb[:, wc, :], hT[:, wc, :],
               start=(wc == 0), stop=(wc == WC - 1))
        # LeakyReLU(y1 + b1): parametric relu on the ACT engine
        lT = pt_pool.tile([128, R], cd, tag="lT")
        nc.scalar.activation(lT, ps_y1, mybir.ActivationFunctionType.Prelu,
                             bias=b1_sb[:, fc:fc + 1], scale=1.0, alpha=LEAKY)
        for qt in range(RT):
            mm(ps_y2[:, qt, :], lT[:, qt * 128:(qt + 1) * 128], w2_sb,
               start=(fc == 0), stop=(fc == FC - 1))

    # ------------------------------------------ out = LN(y2 + b2 + h) * g + b
    out_re = t["out"].rearrange("(q p) w -> q p w", p=128)
    for qt in range(RT):
        sum2 = stream.tile([128, W], f32, tag="sum")
        nc.vector.tensor_add(sum2, ps_y2[:, qt, :], h_f32[:, qt, :])
        nc.vector.tensor_add(sum2, sum2, b2_rep)
        layer_norm(sum2, sum2)
        nc.sync.dma_start(out=out_re[qt], in_=sum2)

    ctx.close()


_NC_CACHE = {}


def get_nc(mode=MODE):
    if mode not in _NC_CACHE:
        nc = build_nc(mode)
        nc.finalize()
        _NC_CACHE[mode] = nc
    return _NC_CACHE[mode]


def make_in_maps(inputs, mode=MODE):
    import ml_dtypes
    x = np.ascontiguousarray(inputs["x"], dtype=np.float32)
    names = ["w_q_m", "w_k_m", "w_v_m", "w_q_c", "w_k_c", "w_v_c",
             "w_o", "ln_g", "ln_b", "ffn_w1", "ffn_b1", "ffn_w2", "ffn_b2"]
    shared = {}
    for n in names:
        a = np.ascontiguousarray(inputs[n], dtype=np.float32)
        if mode == "bf16" and n in CAST_NAMES:
            a = a.astype(ml_dtypes.bfloat16)
        shared[n] = a
    in_maps = []
    for c in range(NCORES):
        m = dict(shared)
        m["x_rows"] = np.ascontiguousarray(x[c * R:(c + 1) * R])
        in_maps.append(m)
    return in_maps


def kernel(**inputs):
    in_maps = make_in_maps(inputs)
    nc = get_nc()
    res = run_bass_kernel_spmd(nc, in_maps, list(range(NCORES)))
    return np.concatenate([res.results[c]["out"] for c in range(NCORES)], axis=0)
